# revision 10
# baseline (speedup 1.0000x reference)
"""Conv2d 3x3 (stride 1, pad 1) as Bass/Tile kernel for Trainium2, 8 cores.

Problem: x [32,128,56,56] f32, weight [256,128,3,3] f32
         -> out [32,256,56,56] f32  (cross-correlation, NCHW/OIHW)

Strategy:
  - Data parallel: 4 images per core across 8 NeuronCores.
  - Conv = sum over 9 kernel taps of a [Cin=128] x [Cout] matmul applied to
    shifted views of the zero-padded input. Cin=128 is the contraction
    (partition) dim; PSUM accumulates the 9 taps.
  - Host zero-pads x to [.,.,58,58]; shifted views are pure access-pattern
    arithmetic on the SBUF-resident padded image.
  - dtype float32r (TF32-like matmul mode): full matmul throughput at
    N>=256 moving elements, ~1.5e-4 relative error.
  - Output tiled [co_tile(2) x row_block(7)]: each PSUM tile is
    [128 cout, 8 rows x 56 cols = 448] (fits one PSUM bank).
"""

import numpy as np

B, CIN, H, W = 32, 128, 56, 56
COUT = 256
HP, WP = H + 2, W + 2  # padded
NCORES = 8
BPC = B // NCORES  # images per core
RB = 8  # output rows per PSUM tile
NRB = H // RB  # 7 row blocks
NFREE = RB * W  # 448 moving elements per matmul
NCO = COUT // 128  # 2 cout tiles

_cache = {}


def _build(reps: int = 1):
    import concourse.mybir as mybir
    import concourse.tile as tile
    from concourse import bacc

    nc = bacc.Bacc("TRN2", target_bir_lowering=False)
    f32r = mybir.dt.float32r
    f32 = mybir.dt.float32

    x = nc.dram_tensor("x", [BPC, CIN, HP, WP], f32r, kind="ExternalInput")
    w = nc.dram_tensor("w", [CIN, 9 * COUT], f32r, kind="ExternalInput")
    out = nc.dram_tensor("out", [BPC, NCO, 128, NRB, NFREE], f32,
                         kind="ExternalOutput")

    with tile.TileContext(nc) as tc:
        with (
            tc.tile_pool(name="wpool", bufs=1) as wpool,
            tc.tile_pool(name="xpool", bufs=1) as xpool,
            tc.tile_pool(name="opool", bufs=4) as opool,
            tc.tile_pool(name="psum", bufs=8, space="PSUM") as pspool,
        ):
            for rep in range(reps):
                wt = wpool.tile([CIN, 9 * COUT], f32r, name="wt", tag="wt")
                # split weight DMA per tap so the first matmul's weights land
                # early instead of waiting for the full 2.3MB transfer
                for t in range(9):
                    nc.sync.dma_start(out=wt[:, t * COUT:(t + 1) * COUT],
                                      in_=w[:, t * COUT:(t + 1) * COUT])

                xts = []
                for b in range(BPC):
                    xt = xpool.tile([CIN, HP, WP], f32r, name="xpad",
                                    tag=f"xpad{b}")
                    # split each image load so compute can start after the
                    # first half arrives
                    nc.sync.dma_start(out=xt[:, 0:HP // 2, :],
                                      in_=x[b, :, 0:HP // 2, :])
                    nc.sync.dma_start(out=xt[:, HP // 2:, :],
                                      in_=x[b, :, HP // 2:, :])
                    xts.append(xt)

                for b in range(BPC):
                    for co in range(NCO):
                        for r in range(NRB):
                            ps = pspool.tile([128, NFREE], f32, name="ps",
                                             tag="ps")
                            for kh in range(3):
                                for kw in range(3):
                                    first = kh == 0 and kw == 0
                                    last = kh == 2 and kw == 2
                                    wofs = (kh * 3 + kw) * COUT + co * 128
                                    ws = wt[:, wofs:wofs + 128]
                                    rhs = xts[b][:, r * RB + kh:r * RB + kh + RB,
                                                 kw:kw + W]
                                    nc.tensor.matmul(ps, ws, rhs,
                                                     start=first, stop=last)
                            ot = opool.tile([128, NFREE], f32)
                            nc.vector.tensor_copy(ot, ps)
                            nc.sync.dma_start(out=out[b, co, :, r, :], in_=ot)
    nc.finalize()
    return nc


def _get_runner(reps: int = 1):
    """Build (once) a cached jitted SPMD callable: (xpad_global, wT_global) -> out_global.

    xpad_global: [B, CIN, HP, WP] f32 (batch-sharded over 8 cores)
    wT_global:   [NCORES*CIN, 9*COUT] f32 (replicated: each core gets its copy)
    out_global:  [B, NCO, 128, NRB, NFREE] f32
    """
    key = ("runner", reps)
    if key in _cache:
        return _cache[key]

    import jax
    import jax.numpy as jnp
    from jax.experimental.shard_map import shard_map
    from jax.sharding import Mesh, NamedSharding, PartitionSpec
    from concourse.bass2jax import (
        _bass_exec_p,
        install_neuronx_cc_hook,
        partition_id_tensor,
    )

    nc = _build(reps)
    install_neuronx_cc_hook()

    in_names = ["x", "w", "out"]
    out_names = ["out"]
    out_shape = (BPC, NCO, 128, NRB, NFREE)
    out_avals = (jax.core.ShapedArray(out_shape, np.float32),)
    if nc.partition_id_tensor is not None:
        in_names = in_names + [nc.partition_id_tensor.name]

    def _body(xs, ws, zs):
        operands = [xs, ws, zs]
        if nc.partition_id_tensor is not None:
            operands.append(partition_id_tensor())
        outs = _bass_exec_p.bind(
            *operands,
            out_avals=tuple(out_avals),
            in_names=tuple(in_names),
            out_names=tuple(out_names),
            lowering_input_output_aliases=(),
            sim_require_finite=True,
            sim_require_nnan=True,
            nc=nc,
        )
        return outs[0]

    devices = jax.devices()[:NCORES]
    mesh = Mesh(np.asarray(devices), ("core",))
    spec = PartitionSpec("core")
    sharded = jax.jit(
        shard_map(
            _body,
            mesh=mesh,
            in_specs=(spec, spec, spec),
            out_specs=spec,
            check_rep=False,
        ),
        donate_argnums=(2,),
        keep_unused=True,
    )
    zeros_fn = jax.jit(
        lambda: jnp.zeros((B, NCO, 128, NRB, NFREE), np.float32),
        out_shardings=NamedSharding(mesh, spec),
    )
    x_sharding = NamedSharding(mesh, spec)
    _cache[key] = (sharded, zeros_fn, x_sharding)
    return _cache[key]


def kernel(x: np.ndarray, weight: np.ndarray) -> np.ndarray:
    sharded, zeros_fn, x_sharding = _get_runner()

    x = np.ascontiguousarray(x, dtype=np.float32)
    weight = np.ascontiguousarray(weight, dtype=np.float32)

    xpad = np.zeros((B, CIN, HP, WP), dtype=np.float32)
    xpad[:, :, 1:1 + H, 1:1 + W] = x
    # weight [co, ci, kh, kw] -> [ci, (kh kw co)], replicated per core
    wT = np.ascontiguousarray(weight.transpose(1, 2, 3, 0)).reshape(CIN, 9 * COUT)
    wG = np.broadcast_to(wT, (NCORES, CIN, 9 * COUT)).reshape(NCORES * CIN, 9 * COUT)

    out = sharded(xpad, wG, zeros_fn())
    return np.asarray(out).reshape(B, COUT, H, W)


# revision 11
# speedup vs baseline: 29203.9805x; 29203.9805x over previous
"""Conv2d 3x3 (stride 1, pad 1) as Bass/Tile kernel for Trainium2, 8 cores.

Problem: x [32,128,56,56] f32, weight [256,128,3,3] f32
         -> out [32,256,56,56] f32  (cross-correlation, NCHW/OIHW)

Strategy:
  - Data parallel: 4 images per core across 8 NeuronCores.
  - Conv = sum over 9 kernel taps of a [Cin=128] x [Cout] matmul applied to
    shifted views of the zero-padded input. Cin=128 is the contraction
    (partition) dim; PSUM accumulates the 9 taps.
  - Host zero-pads x to [.,.,58,58]; shifted views are pure access-pattern
    arithmetic on the SBUF-resident padded image.
  - dtype float32r (TF32-like matmul mode): full matmul throughput at
    N>=256 moving elements, ~1.5e-4 relative error.
  - Output tiled [co_tile(2) x row_block(7)]: each PSUM tile is
    [128 cout, 8 rows x 56 cols = 448] (fits one PSUM bank).
"""

import numpy as np

B, CIN, H, W = 32, 128, 56, 56
COUT = 256
HP, WP = H + 2, W + 2  # padded
NCORES = 8
BPC = B // NCORES  # images per core
RB = 8  # output rows per PSUM tile
NRB = H // RB  # 7 row blocks
NFREE = RB * W  # 448 moving elements per matmul
NCO = COUT // 128  # 2 cout tiles

_cache = {}


def _emit_rep(nc, x, w, out, wpool, xpool, opool, pspool, mybir):
    """One full conv pass over this core's 4 images."""
    f32r = mybir.dt.float32r
    f32 = mybir.dt.float32

    wt = wpool.tile([CIN, 9 * COUT], f32r, name="wt", tag="wt")
    # split weight DMA per tap so the first matmul's weights land early
    # instead of waiting for the full 2.3MB transfer
    for t in range(9):
        nc.sync.dma_start(out=wt[:, t * COUT:(t + 1) * COUT],
                          in_=w[:, t * COUT:(t + 1) * COUT])

    xts = []
    for b in range(BPC):
        xt = xpool.tile([CIN, HP, WP], f32r, name="xpad", tag=f"xpad{b}")
        # split each image load so compute can start after the first half
        nc.sync.dma_start(out=xt[:, 0:HP // 2, :], in_=x[b, :, 0:HP // 2, :])
        nc.sync.dma_start(out=xt[:, HP // 2:, :], in_=x[b, :, HP // 2:, :])
        xts.append(xt)

    for b in range(BPC):
        for co in range(NCO):
            for r in range(NRB):
                ps = pspool.tile([128, NFREE], f32, name="ps", tag="ps")
                for kh in range(3):
                    for kw in range(3):
                        first = kh == 0 and kw == 0
                        last = kh == 2 and kw == 2
                        wofs = (kh * 3 + kw) * COUT + co * 128
                        ws = wt[:, wofs:wofs + 128]
                        rhs = xts[b][:, r * RB + kh:r * RB + kh + RB,
                                     kw:kw + W]
                        nc.tensor.matmul(ps, ws, rhs, start=first, stop=last)
                ot = opool.tile([128, NFREE], f32, name="ot", tag="ot")
                nc.vector.tensor_copy(ot, ps)
                nc.sync.dma_start(out=out[b, co, :, r, :], in_=ot)


def _build(reps: int = 1, loop_n: int = 0):
    """loop_n > 0 wraps the body in a For_i hardware loop (for timing)."""
    import concourse.mybir as mybir
    import concourse.tile as tile
    from concourse import bacc

    nc = bacc.Bacc("TRN2", target_bir_lowering=False)
    f32r = mybir.dt.float32r
    f32 = mybir.dt.float32

    x = nc.dram_tensor("x", [BPC, CIN, HP, WP], f32r, kind="ExternalInput")
    w = nc.dram_tensor("w", [CIN, 9 * COUT], f32r, kind="ExternalInput")
    out = nc.dram_tensor("out", [BPC, NCO, 128, NRB, NFREE], f32,
                         kind="ExternalOutput")

    with tile.TileContext(nc) as tc:
        with (
            tc.tile_pool(name="wpool", bufs=1) as wpool,
            tc.tile_pool(name="xpool", bufs=1) as xpool,
            tc.tile_pool(name="opool", bufs=4) as opool,
            tc.tile_pool(name="psum", bufs=8, space="PSUM") as pspool,
        ):
            pools = (wpool, xpool, opool, pspool)
            if loop_n > 0:
                with tc.For_i(0, loop_n, 1):
                    _emit_rep(nc, x, w, out, *pools, mybir)
            else:
                for _ in range(reps):
                    _emit_rep(nc, x, w, out, *pools, mybir)
    nc.finalize()
    return nc


def _get_runner(reps: int = 1, loop_n: int = 0):
    """Build (once) a cached jitted SPMD callable: (xpad_global, wT_global) -> out_global.

    xpad_global: [B, CIN, HP, WP] f32 (batch-sharded over 8 cores)
    wT_global:   [NCORES*CIN, 9*COUT] f32 (replicated: each core gets its copy)
    out_global:  [B, NCO, 128, NRB, NFREE] f32
    """
    key = ("runner", reps, loop_n)
    if key in _cache:
        return _cache[key]

    import jax
    import jax.numpy as jnp
    from jax.experimental.shard_map import shard_map
    from jax.sharding import Mesh, NamedSharding, PartitionSpec
    from concourse.bass2jax import (
        _bass_exec_p,
        install_neuronx_cc_hook,
        partition_id_tensor,
    )

    nc = _build(reps, loop_n)
    install_neuronx_cc_hook()

    in_names = ["x", "w", "out"]
    out_names = ["out"]
    out_shape = (BPC, NCO, 128, NRB, NFREE)
    out_avals = (jax.core.ShapedArray(out_shape, np.float32),)
    if nc.partition_id_tensor is not None:
        in_names = in_names + [nc.partition_id_tensor.name]

    def _body(xs, ws, zs):
        operands = [xs, ws, zs]
        if nc.partition_id_tensor is not None:
            operands.append(partition_id_tensor())
        outs = _bass_exec_p.bind(
            *operands,
            out_avals=tuple(out_avals),
            in_names=tuple(in_names),
            out_names=tuple(out_names),
            lowering_input_output_aliases=(),
            sim_require_finite=True,
            sim_require_nnan=True,
            nc=nc,
        )
        return outs[0]

    devices = jax.devices()[:NCORES]
    mesh = Mesh(np.asarray(devices), ("core",))
    spec = PartitionSpec("core")
    sharded = jax.jit(
        shard_map(
            _body,
            mesh=mesh,
            in_specs=(spec, spec, spec),
            out_specs=spec,
            check_rep=False,
        ),
        donate_argnums=(2,),
        keep_unused=True,
    )
    zeros_fn = jax.jit(
        lambda: jnp.zeros((B, NCO, 128, NRB, NFREE), np.float32),
        out_shardings=NamedSharding(mesh, spec),
    )
    x_sharding = NamedSharding(mesh, spec)
    _cache[key] = (sharded, zeros_fn, x_sharding)
    return _cache[key]


def kernel(x: np.ndarray, weight: np.ndarray) -> np.ndarray:
    sharded, zeros_fn, x_sharding = _get_runner()

    x = np.ascontiguousarray(x, dtype=np.float32)
    weight = np.ascontiguousarray(weight, dtype=np.float32)

    xpad = np.zeros((B, CIN, HP, WP), dtype=np.float32)
    xpad[:, :, 1:1 + H, 1:1 + W] = x
    # weight [co, ci, kh, kw] -> [ci, (kh kw co)], replicated per core
    wT = np.ascontiguousarray(weight.transpose(1, 2, 3, 0)).reshape(CIN, 9 * COUT)
    wG = np.broadcast_to(wT, (NCORES, CIN, 9 * COUT)).reshape(NCORES * CIN, 9 * COUT)

    out = sharded(xpad, wG, zeros_fn())
    return np.asarray(out).reshape(B, COUT, H, W)
